# revision 4
# baseline (speedup 1.0000x reference)
"""Trainium2 Bass kernel for MinibatchDiscrimination.

Reference computation (B=256, IN=1024, O=64, K=50):
    M = (x @ T).reshape(B, O, K)
    l1[i,j,o] = sum_k |M[i,o,k] - M[j,o,k]|
    out = concat([x, sum_j exp(-l1) - 1], axis=1)          # [B, IN + O]

Sharding: the O (out_features) dimension is split across the 8 NeuronCores
(8 features per core); x is replicated. Each core computes its [256, 8]
feature block; the host gathers the blocks and concatenates with x.

Per-core layout (K padded 50 -> 64 with zero columns so the k-reduction is a
clean power of two; zero pads contribute |0-0| = 0 to the L1 sum):
  1. PE matmul: M[256, 512] = xT.T @ T_local   (bf16 inputs, f32 PSUM)
  2. M cast to bf16 (the canonical value used on BOTH sides of the pairwise
     subtraction so the diagonal distance is exactly zero), PE-transposed and
     staged to DRAM as MT[(o,k), j].
  3. Per o: MT block broadcast-DMA'd to all 128 partitions (mrep), then for
     each k one fused DVE tensor_scalar computes |M[j,o,k] - M[i,o,k]| for
     128 i (partitions) x 256 j (free), writing Dabs[i, (k,j)].
  4. Binary-tree tensor_tensor adds reduce the 64 k-slices -> l1[i, j].
  5. ScalarE activation computes exp(-l1) (scale=-1), DVE reduce sums over j,
     and a final -1.0 gives the feature column.
"""

import numpy as np
import ml_dtypes

B = 256
IN_FEATURES = 1024
O_TOTAL = 64
K = 50
K64 = 64
N_CORES = 8
O_LOC = O_TOTAL // N_CORES          # 8 features per core
N_LOC = O_LOC * K64                 # 512 padded M columns per core
P = 128                             # partitions
ITILES = B // P                     # 2 row tiles
CC = IN_FEATURES // P               # 8 contraction chunks

_cache = {}


def _build_program():
    import concourse.mybir as mybir
    from concourse import bacc, tile
    from concourse.masks import make_identity

    f32 = mybir.dt.float32
    bf16 = mybir.dt.bfloat16
    Alu = mybir.AluOpType
    Act = mybir.ActivationFunctionType

    nc = bacc.Bacc("TRN2", target_bir_lowering=False, debug=False,
                   enable_asserts=False)

    xT_d = nc.dram_tensor("xT", [IN_FEATURES, B], bf16, kind="ExternalInput").ap()
    T_d = nc.dram_tensor("Tl", [IN_FEATURES, N_LOC], bf16, kind="ExternalInput").ap()
    feat_d = nc.dram_tensor("feat", [B, O_LOC], f32, kind="ExternalOutput").ap()

    # binary-tree region offsets inside the Dabs scratch tile (bf16 elems)
    tree_off = [0]
    sz = K64 * B                    # 16384
    for _ in range(7):
        tree_off.append(tree_off[-1] + sz)
        sz //= 2
    dabs_cols = tree_off[6] + B     # 32512

    with tile.TileContext(nc) as tc:
        with (
            tc.tile_pool(name="static", bufs=1) as static,
            tc.tile_pool(name="mtp", bufs=2) as mtp,
            tc.tile_pool(name="mrepp", bufs=2) as mrepp,
            tc.tile_pool(name="dabsp", bufs=1) as dabsp,
            tc.tile_pool(name="expp", bufs=2) as expp,
            tc.tile_pool(name="pmp", bufs=2, space="PSUM") as pmp,
            tc.tile_pool(name="tpp", bufs=2, space="PSUM") as tpp,
            tc.tile_pool(name="dramp", bufs=1, space="DRAM") as dramp,
        ):
            # ---- load inputs ------------------------------------------------
            xt_sb = static.tile([P, CC * B], bf16, tag="xt")
            t_sb = static.tile([P, CC * N_LOC], bf16, tag="t")
            for cc in range(CC):
                nc.sync.dma_start(out=xt_sb[:, cc * B:(cc + 1) * B],
                                  in_=xT_d[cc * P:(cc + 1) * P, :])
                nc.sync.dma_start(out=t_sb[:, cc * N_LOC:(cc + 1) * N_LOC],
                                  in_=T_d[cc * P:(cc + 1) * P, :])

            ident = static.tile([P, P], bf16, tag="ident")
            make_identity(nc, ident[:, :])

            # ---- M = x @ T_local -------------------------------------------
            m_bf = []     # [128, 512] bf16 per itile (canonical rounded M)
            m_sc = []     # [128, 512] f32 per itile (same values, for AP scalars)
            for it in range(ITILES):
                pm = pmp.tile([P, N_LOC], f32, tag="pm")
                for cc in range(CC):
                    nc.tensor.matmul(
                        pm[:, :],
                        lhsT=xt_sb[:, cc * B + it * P: cc * B + it * P + P],
                        rhs=t_sb[:, cc * N_LOC:(cc + 1) * N_LOC],
                        start=(cc == 0), stop=(cc == CC - 1),
                    )
                mb = static.tile([P, N_LOC], bf16, tag=f"mbf{it}")
                nc.scalar.copy(mb[:, :], pm[:, :])
                ms = static.tile([P, N_LOC], f32, tag=f"msc{it}")
                nc.scalar.copy(ms[:, :], mb[:, :])
                m_bf.append(mb)
                m_sc.append(ms)

            # ---- MT[(o,k), j] staged to DRAM -------------------------------
            mt_d = dramp.tile([N_LOC, B], bf16, tag="mt")
            for o in range(O_LOC):
                mts = mtp.tile([K64, B], bf16, tag="mts")
                for it in range(ITILES):
                    tp = tpp.tile([K64, P], bf16, tag="tp")
                    nc.tensor.transpose(
                        tp[:, :], m_bf[it][:, o * K64:(o + 1) * K64], ident[:, :])
                    nc.scalar.copy(mts[:, it * P:(it + 1) * P], tp[:, :])
                nc.sync.dma_start(out=mt_d[o * K64:(o + 1) * K64, :],
                                  in_=mts[:, :])

            # ---- pairwise L1 + exp + reduce --------------------------------
            feat_sb = [static.tile([P, O_LOC], f32, tag=f"feat{it}",
                                   name=f"feat{it}")
                       for it in range(ITILES)]
            for o in range(O_LOC):
                mrep = mrepp.tile([P, K64 * B], bf16, tag="mrep")
                src = mt_d[o * K64:(o + 1) * K64, :].rearrange("r c -> (r c)")
                nc.sync.dma_start(out=mrep[:, :],
                                  in_=src.partition_broadcast(P))
                for it in range(ITILES):
                    dabs = dabsp.tile([P, dabs_cols], bf16, tag="dabs")
                    # d = mrep - M[i] (per-partition scalar), one instr per k
                    for k in range(K64):
                        nc.vector.tensor_scalar(
                            out=dabs[:, k * B:(k + 1) * B],
                            in0=mrep[:, k * B:(k + 1) * B],
                            scalar1=m_sc[it][:, o * K64 + k: o * K64 + k + 1],
                            scalar2=None,
                            op0=Alu.subtract,
                        )
                    # |d| = max(-d, d), one fused DVE op over the whole block
                    nc.vector.scalar_tensor_tensor(
                        out=dabs[:, :K64 * B], in0=dabs[:, :K64 * B],
                        scalar=-1.0, in1=dabs[:, :K64 * B],
                        op0=Alu.mult, op1=Alu.max)
                    # binary tree reduction over k: 64 -> 1 slices of [128, 256]
                    for lvl in range(6):
                        half = (K64 * B) >> (lvl + 1)
                        nc.vector.tensor_tensor(
                            out=dabs[:, tree_off[lvl + 1]:tree_off[lvl + 1] + half],
                            in0=dabs[:, tree_off[lvl]:tree_off[lvl] + half],
                            in1=dabs[:, tree_off[lvl] + half:tree_off[lvl] + 2 * half],
                            op=Alu.add,
                        )
                    ex = expp.tile([P, B], f32, tag="ex")
                    nc.scalar.activation(
                        out=ex[:, :],
                        in_=dabs[:, tree_off[6]:tree_off[6] + B],
                        func=Act.Exp, scale=-1.0)
                    nc.vector.tensor_reduce(
                        out=feat_sb[it][:, o:o + 1], in_=ex[:, :],
                        axis=mybir.AxisListType.X, op=Alu.add)

            for it in range(ITILES):
                nc.vector.tensor_scalar(
                    out=feat_sb[it][:, :], in0=feat_sb[it][:, :],
                    scalar1=1.0, scalar2=None, op0=Alu.subtract)
                nc.sync.dma_start(out=feat_d[it * P:(it + 1) * P, :],
                                  in_=feat_sb[it][:, :])

    nc.compile()
    return nc


def _get_program():
    if "nc" not in _cache:
        _cache["nc"] = _build_program()
    return _cache["nc"]


def prepare_in_maps(x, T):
    """Host-side sharding: transpose/cast x, slice + K-pad T per core."""
    bf = ml_dtypes.bfloat16
    xT = np.ascontiguousarray(np.asarray(x, dtype=np.float32).T).astype(bf)
    Tf = np.asarray(T, dtype=np.float32)
    in_maps = []
    for c in range(N_CORES):
        Tl = np.zeros((IN_FEATURES, N_LOC), dtype=bf)
        for o in range(O_LOC):
            g = (c * O_LOC + o) * K
            Tl[:, o * K64: o * K64 + K] = Tf[:, g: g + K].astype(bf)
        in_maps.append({"xT": xT, "Tl": Tl})
    return in_maps


def run_cores(in_maps, trace=False, tmpdir=None):
    from concourse import bass_utils
    nc = _get_program()
    return bass_utils.run_bass_kernel_spmd(
        nc, in_maps, core_ids=list(range(N_CORES)), trace=trace, tmpdir=tmpdir)


def kernel(x, T):
    x = np.asarray(x, dtype=np.float32)
    res = run_cores(prepare_in_maps(x, T))
    feat = np.concatenate(
        [res.results[c]["feat"].astype(np.float32) for c in range(N_CORES)],
        axis=1)
    return np.concatenate([x, feat], axis=1)


# revision 12
# speedup vs baseline: 2.1377x; 2.1377x over previous
"""Trainium2 Bass kernel for MinibatchDiscrimination.

Reference computation (B=256, IN=1024, O=64, K=50):
    M = (x @ T).reshape(B, O, K)
    l1[i,j,o] = sum_k |M[i,o,k] - M[j,o,k]|
    out = concat([x, sum_j exp(-l1) - 1], axis=1)          # [B, IN + O]

Sharding: the O (out_features) dimension is split across the 8 NeuronCores
(8 features per core); x is replicated. Each core computes its [256, 8]
feature block; the host gathers the blocks and concatenates with x.

Per-core pipeline (K padded 50 -> 64 with zero columns; pads add |0-0| = 0):
  1. PE matmul: M[256, 512] = xT.T @ T_local (bf16 in, f32 PSUM), cast to
     bf16 (the canonical value used on BOTH sides of the pairwise
     subtraction so the diagonal distance is exactly zero). -M is staged to
     DRAM.
  2. All-pairs signed differences are generated by the PE with an affine
     matmul: for each (itile, o) block,
        diff[i, (j,k)] = sum_p lhsT[p, i] * rhs[p, (j,k)]
     where lhsT = [M_o^T (64 k-rows) ; ones] and rhs = [I64 tiled over j ;
     -M_o row]. Output lands in PSUM as [128, (j,k)] f32 chunks.
  3. Each PSUM chunk takes one of two abs+k-reduce paths (balancing DVE and
     ScalarE):  (a) DVE tensor_reduce(add, apply_absolute_value) straight
     from PSUM, or (b) ScalarE Abs-cast to bf16 SBUF + DVE binary-tree
     tensor_tensor adds at 2x.
  4. ScalarE exp(-l1) (scale=-1), DVE reduce over j, -1.0, DMA out.
"""

import numpy as np
import ml_dtypes

B = 256
IN_FEATURES = 1024
O_TOTAL = 64
K = 50
K64 = 64
N_CORES = 8
O_LOC = O_TOTAL // N_CORES          # 8 features per core
N_LOC = O_LOC * K64                 # 512 padded M columns per core
P = 128                             # partitions
ITILES = B // P                     # 2 row tiles
CC = IN_FEATURES // P               # 8 contraction chunks
JCHUNK = 32                         # j's per PSUM chunk (32*64 = 2048 f32)
NCHUNK = B // JCHUNK                # 8 chunks per (itile, o) block
DIRECT_EVERY = 6                    # every Nth chunk takes the DVE-direct path

_cache = {}


def _build_program():
    import concourse.mybir as mybir
    from concourse import bacc, tile
    from concourse.masks import make_identity

    f32 = mybir.dt.float32
    bf16 = mybir.dt.bfloat16
    Alu = mybir.AluOpType
    Act = mybir.ActivationFunctionType

    nc = bacc.Bacc("TRN2", target_bir_lowering=False, debug=False,
                   enable_asserts=False)

    xT_d = nc.dram_tensor("xT", [IN_FEATURES, B], bf16, kind="ExternalInput").ap()
    T_d = nc.dram_tensor("Tl", [IN_FEATURES, N_LOC], bf16, kind="ExternalInput").ap()
    feat_d = nc.dram_tensor("feat", [B, O_LOC], f32, kind="ExternalOutput").ap()

    JK = K64 * B                    # 16384 columns per (itile, o) block
    CH = JCHUNK * K64               # 2048 elements per chunk

    with tile.TileContext(nc) as tc:
        with (
            tc.tile_pool(name="static", bufs=1) as static,
            tc.tile_pool(name="babsp", bufs=3) as babsp,
            tc.tile_pool(name="dexpp", bufs=2) as dexpp,
            tc.tile_pool(name="etp", bufs=2) as etp,
            tc.tile_pool(name="dramp", bufs=1, space="DRAM") as dramp,
        ):
            # ---- stage 1: load inputs, M = x @ T_local ---------------------
            xt_sb = static.tile([P, CC * B], bf16, tag="xt")
            t_sb = static.tile([P, CC * N_LOC], bf16, tag="t")
            for cc in range(CC):
                nc.sync.dma_start(out=xt_sb[:, cc * B:(cc + 1) * B],
                                  in_=xT_d[cc * P:(cc + 1) * P, :])
                nc.sync.dma_start(out=t_sb[:, cc * N_LOC:(cc + 1) * N_LOC],
                                  in_=T_d[cc * P:(cc + 1) * P, :])

            ident = static.tile([P, P], bf16, tag="ident")
            make_identity(nc, ident[:, :])

            negm_d = dramp.tile([B, N_LOC], bf16, tag="negm_d")
            m_bf = []
            with tc.tile_pool(name="mmp", bufs=2, space="PSUM") as mmp:
                for it in range(ITILES):
                    pm = mmp.tile([P, N_LOC], f32, tag="pm")
                    for cc in range(CC):
                        nc.tensor.matmul(
                            pm[:, :],
                            lhsT=xt_sb[:, cc * B + it * P: cc * B + it * P + P],
                            rhs=t_sb[:, cc * N_LOC:(cc + 1) * N_LOC],
                            start=(cc == 0), stop=(cc == CC - 1),
                        )
                    mb = static.tile([P, N_LOC], bf16, tag=f"mbf{it}",
                                     name=f"mbf{it}")
                    nc.scalar.copy(mb[:, :], pm[:, :])
                    m_bf.append(mb)
                    ng = static.tile([P, N_LOC], bf16, tag=f"neg{it}",
                                     name=f"neg{it}")
                    nc.vector.tensor_scalar(out=ng[:, :], in0=mb[:, :],
                                            scalar1=-1.0, scalar2=None,
                                            op0=Alu.mult)
                    nc.sync.dma_start(out=negm_d[it * P:(it + 1) * P, :],
                                      in_=ng[:, :])

            # ---- stage 2: lhsT tiles  [M_o^T ; ones] -----------------------
            lhs = []
            with tc.tile_pool(name="tpp", bufs=2, space="PSUM") as tpp:
                for o in range(O_LOC):
                    lt = static.tile([K64 + 1, B], bf16, tag=f"lhs{o}",
                                     name=f"lhs{o}")
                    for it in range(ITILES):
                        tp = tpp.tile([K64, P], bf16, tag="tp")
                        nc.tensor.transpose(
                            tp[:, :], m_bf[it][:, o * K64:(o + 1) * K64],
                            ident[:, :])
                        nc.scalar.copy(lt[0:K64, it * P:(it + 1) * P], tp[:, :])
                    nc.vector.memset(lt[K64:K64 + 1, :], 1.0)
                    lhs.append(lt)

            # ---- stage 3: rhs tiles [I64 tiled ; -M_o row] (ping-pong) -----
            icon = static.tile([K64, K64], bf16, tag="icon")
            make_identity(nc, icon[:, :])
            rhs_t = []
            for h in range(2):
                rt = static.tile([K64 + 1, JK], bf16, tag=f"rhs{h}",
                                 name=f"rhs{h}")
                src = icon[:, :].rearrange("p (j k) -> p j k", j=1).\
                    broadcast_to([K64, B, K64])
                nc.sync.dma_start(out=rt[0:K64, :].rearrange(
                    "p (j k) -> p j k", k=K64), in_=src)
                rhs_t.append(rt)

            # ---- stage 4: per (o, itile): diffs -> |.| -> k-sum -> exp -----
            feat_sb = [static.tile([P, O_LOC], f32, tag=f"feat{it}",
                                   name=f"feat{it}")
                       for it in range(ITILES)]
            chunk_idx = 0
            stage4 = tc.tile_pool(name="chp", bufs=2, space="PSUM")
            chp = stage4.__enter__()
            for o in range(O_LOC):
                rt = rhs_t[o % 2]
                nc.sync.dma_start(
                    out=rt[K64:K64 + 1, :].rearrange("p (j k) -> p j k",
                                                     k=K64),
                    in_=negm_d[:, o * K64:(o + 1) * K64].rearrange(
                        "(o j) k -> o j k", o=1))
                for it in range(ITILES):
                    dexp = dexpp.tile([P, B], f32, tag="dexp")
                    for c in range(NCHUNK):
                        ch = chp.tile([P, CH], f32, tag="ch")
                        for q in range(CH // 512):
                            col = c * CH + q * 512
                            nc.tensor.matmul(
                                ch[:, q * 512:(q + 1) * 512],
                                lhsT=lhs[o][:, it * P:(it + 1) * P],
                                rhs=rt[:, col:col + 512],
                                start=True, stop=True)
                        dslice = dexp[:, c * JCHUNK:(c + 1) * JCHUNK]
                        if chunk_idx % DIRECT_EVERY == 0:
                            # DVE path: fused |.| + k-reduce from PSUM
                            nc.vector.tensor_reduce(
                                out=dslice,
                                in_=ch[:, :].rearrange("p (j k) -> p j k",
                                                       k=K64),
                                axis=mybir.AxisListType.X, op=Alu.add,
                                apply_absolute_value=True)
                        else:
                            # ScalarE |.| cast to bf16, DVE 2x tree reduce
                            ba = babsp.tile([P, 2 * CH], bf16, tag="ba")
                            nc.scalar.activation(out=ba[:, 0:CH],
                                                 in_=ch[:, :], func=Act.Abs)
                            ofs, width = 0, K64
                            while width > 1:
                                half = width // 2
                                nxt = ofs + JCHUNK * width
                                src3 = ba[:, ofs:ofs + JCHUNK * width].\
                                    rearrange("p (j k) -> p j k", k=width)
                                if half > 1:
                                    dst = ba[:, nxt:nxt + JCHUNK * half]
                                    nc.vector.tensor_tensor(
                                        out=dst.rearrange(
                                            "p (j k) -> p j k", k=half),
                                        in0=src3[:, :, 0:half],
                                        in1=src3[:, :, half:width],
                                        op=Alu.add)
                                else:
                                    nc.vector.tensor_tensor(
                                        out=dslice.rearrange(
                                            "p (j k) -> p j k", k=1),
                                        in0=src3[:, :, 0:1],
                                        in1=src3[:, :, 1:2],
                                        op=Alu.add)
                                ofs, width = nxt, half
                        chunk_idx += 1
                    et = etp.tile([P, B], f32, tag="et")
                    nc.scalar.activation(out=et[:, :], in_=dexp[:, :],
                                         func=Act.Exp, scale=-1.0)
                    nc.vector.tensor_reduce(
                        out=feat_sb[it][:, o:o + 1], in_=et[:, :],
                        axis=mybir.AxisListType.X, op=Alu.add)

            for it in range(ITILES):
                nc.vector.tensor_scalar(
                    out=feat_sb[it][:, :], in0=feat_sb[it][:, :],
                    scalar1=1.0, scalar2=None, op0=Alu.subtract)
                nc.sync.dma_start(out=feat_d[it * P:(it + 1) * P, :],
                                  in_=feat_sb[it][:, :])
            stage4.__exit__(None, None, None)

    nc.compile()
    return nc


def _get_program():
    if "nc" not in _cache:
        _cache["nc"] = _build_program()
    return _cache["nc"]


def prepare_in_maps(x, T):
    """Host-side sharding: transpose/cast x, slice + K-pad T per core."""
    bf = ml_dtypes.bfloat16
    xT = np.ascontiguousarray(np.asarray(x, dtype=np.float32).T).astype(bf)
    Tf = np.asarray(T, dtype=np.float32)
    in_maps = []
    for c in range(N_CORES):
        Tl = np.zeros((IN_FEATURES, N_LOC), dtype=bf)
        for o in range(O_LOC):
            g = (c * O_LOC + o) * K
            Tl[:, o * K64: o * K64 + K] = Tf[:, g: g + K].astype(bf)
        in_maps.append({"xT": xT, "Tl": Tl})
    return in_maps


def run_cores(in_maps, trace=False, tmpdir=None):
    from concourse import bass_utils
    nc = _get_program()
    return bass_utils.run_bass_kernel_spmd(
        nc, in_maps, core_ids=list(range(N_CORES)), trace=trace, tmpdir=tmpdir)


def kernel(x, T):
    x = np.asarray(x, dtype=np.float32)
    res = run_cores(prepare_in_maps(x, T))
    feat = np.concatenate(
        [res.results[c]["feat"].astype(np.float32) for c in range(N_CORES)],
        axis=1)
    return np.concatenate([x, feat], axis=1)
